# revision 34
# baseline (speedup 1.0000x reference)
"""AdaptiveTokenMerger Trainium2 kernel (8 NeuronCores, SPMD).

Decomposition:
  Phase 1 (device): per-token score logits. The grouped conv1d is folded
    algebraically into 3 dense shifted matmuls (Ak weight prep on host), so
    phase 1 is pure f32 PE matmuls + exact ReLU. Outputs pre-sigmoid logits.
  Host: sigmoids/quantile via jax-CPU (bitwise-matching the reference's
    boundary decisions), the inherently sequential boundary-enforce walk,
    per-segment softmax scalars, and the block-compaction weight matrices M.
  Phase 2 (device): merged vectors = M^T @ x per 128-token tile (f32 PE
    matmuls, PSUM accumulation across the tile-crossing part). Output is
    block-compacted; host permutes rows into the final (B, L, D) layout.

Sharding: pure data parallel over (batch row, half sequence) = 8 shards.
"""

import numpy as np
from contextlib import ExitStack

B, L, D = 4, 8192, 512
NCORES = 8
HALF = L // 2            # 4096 tokens per shard (plus halo/padding)
NT = 33                  # 128-token x tiles per shard window
TS = NT * 128            # 4224 padded window length
NB = NT - 1              # 32 output blocks per shard
GRP = 384                # phase-1 matmul moving free dim (11 * 384 = 4224)
NGRP = TS // GRP
M1R = 20                 # rows of a tile reachable by a crossing-segment tail
SLOT = 32                # output slots per 128-token tile (hard bound: 128/MIN_PS)
MIN_PS, MAX_PS = 4, 16

_NC_CACHE = {}
PROFILE = False          # set True (e.g. from test.py) to capture NTFF exec times
LAST_EXEC_NS = {}        # phase name -> exec_time_ns (when PROFILE)
LAST_TRACE_DIRS = {}


def _get_jax():
    import jax
    try:
        jax.config.update("jax_platforms", "axon,cpu")
    except Exception:
        pass
    return jax


def _cpu_dev(jax):
    try:
        return jax.devices("cpu")[0]
    except Exception:
        return None


# ---------------------------------------------------------------- builders

def _build_phase1():
    if "p1" in _NC_CACHE:
        return _NC_CACHE["p1"]
    from concourse import bacc, mybir
    import concourse.tile as tile
    import concourse.bass as bass

    f32 = mybir.dt.float32
    f32r = mybir.dt.float32r
    AF = mybir.ActivationFunctionType
    ALU = mybir.AluOpType
    nc = bacc.Bacc(None, target_bir_lowering=False, debug=False)
    # xT / wmats / vecsr arrive pre-rounded to the fp32r grid (host _round_f32r)
    xT = nc.declare_dram_parameter("xT", [D, TS + 2], f32r, False)
    wmats = nc.declare_dram_parameter("wmats", [16, 128, 128], f32r, False)
    vecs = nc.declare_dram_parameter("vecs", [128, 4], f32, False)
    vecsr = nc.declare_dram_parameter("vecsr", [128, 2], f32r, False)
    l1o = nc.declare_dram_parameter("l1", [1, TS], f32, True)
    l2o = nc.declare_dram_parameter("l2", [1, TS], f32, True)

    # token groups: 8 x 512 + tail 128
    GROUPS = [(i * 512, 512) for i in range(8)] + [(4096, 128)]

    with tile.TileContext(nc) as tc:
        with ExitStack() as ctx:
            xtp = ctx.enter_context(tc.tile_pool(name="xt", bufs=1))
            wtp = ctx.enter_context(tc.tile_pool(name="wt", bufs=1))
            hsp = ctx.enter_context(tc.tile_pool(name="hs", bufs=3))
            lsp = ctx.enter_context(tc.tile_pool(name="ls", bufs=3))
            psp = ctx.enter_context(
                tc.tile_pool(name="ps", bufs=3, space=bass.MemorySpace.PSUM))
            pslp = ctx.enter_context(
                tc.tile_pool(name="psl", bufs=1, space=bass.MemorySpace.PSUM))

            vt = wtp.tile([128, 4], f32, tag="vecs")
            nc.sync.dma_start(vt[:], vecs[:, :])
            vr = wtp.tile([128, 2], f32r, tag="vecsr")
            nc.sync.dma_start(vr[:], vecsr[:, :])
            wts = []
            for i in range(16):
                t = wtp.tile([128, 128], f32r, tag=f"w{i}")
                nc.sync.dma_start(t[:], wmats[i, :, :])
                wts.append(t)
            w1t, cct, clt, crt = wts[0:4], wts[4:8], wts[8:12], wts[12:16]
            xts = []
            CSPL = 1538          # first token-chunk of each xT row block
            for kc in range(4):
                t = xtp.tile([128, TS + 2], f32r, tag=f"x{kc}")
                nc.sync.dma_start(t[:, 0:CSPL],
                                  xT[kc * 128:(kc + 1) * 128, 0:CSPL])
                xts.append(t)
            for kc in range(4):
                nc.sync.dma_start(xts[kc][:, CSPL:TS + 2],
                                  xT[kc * 128:(kc + 1) * 128, CSPL:TS + 2])

            h1sall = hsp.tile([128, TS], f32r, tag="h1sall")
            h2sall = hsp.tile([128, TS], f32r, tag="h2sall")
            for o, gs in GROUPS:
                h1p = psp.tile([128, 512], f32, tag="h1")
                for kc in range(4):
                    nc.tensor.matmul(h1p[:, 0:gs], w1t[kc][:],
                                     xts[kc][:, 1 + o:1 + o + gs],
                                     start=(kc == 0), stop=(kc == 3))
                nc.scalar.activation(h1sall[:, o:o + gs], h1p[:, 0:gs],
                                     AF.Relu, bias=vt[:, 2:3])
                h2p = psp.tile([128, 512], f32, tag="h2")
                idx = 0
                for kc in range(4):
                    for mats, off in ((cct, 1), (clt, 0), (crt, 2)):
                        nc.tensor.matmul(h2p[:, 0:gs], mats[kc][:],
                                         xts[kc][:, off + o:off + o + gs],
                                         start=(idx == 0), stop=(idx == 11))
                        idx += 1
                nc.vector.tensor_scalar(h2sall[:, o:o + gs], h2p[:, 0:gs],
                                        vt[:, 3:4], 0.0, ALU.add, ALU.max)
            for o, gs in GROUPS:
                l1p = pslp.tile([1, 512], f32, tag="l1p")
                nc.tensor.matmul(l1p[:, 0:gs], vr[:, 0:1], h1sall[:, o:o + gs],
                                 start=True, stop=True)
                l1t = lsp.tile([1, 512], f32, tag="l1t")
                nc.vector.tensor_copy(l1t[:, 0:gs], l1p[:, 0:gs])
                nc.gpsimd.dma_start(l1o[:, o:o + gs], l1t[:, 0:gs])
                l2p = pslp.tile([1, 512], f32, tag="l2p")
                nc.tensor.matmul(l2p[:, 0:gs], vr[:, 1:2], h2sall[:, o:o + gs],
                                 start=True, stop=True)
                l2t = lsp.tile([1, 512], f32, tag="l2t")
                nc.scalar.activation(l2t[:, 0:gs], l2p[:, 0:gs], AF.Copy)
                nc.gpsimd.dma_start(l2o[:, o:o + gs], l2t[:, 0:gs])

    nc.finalize()
    _NC_CACHE["p1"] = nc
    return nc


def _build_phase2():
    if "p2" in _NC_CACHE:
        return _NC_CACHE["p2"]
    from concourse import bacc, mybir
    import concourse.tile as tile
    import concourse.bass as bass

    f32 = mybir.dt.float32
    f32r = mybir.dt.float32r
    AF = mybir.ActivationFunctionType
    nc = bacc.Bacc(None, target_bir_lowering=False, debug=False)
    # host supplies pre-transposed, fp32r-rounded layouts so every DMA is a
    # plain 2D partition-contiguous copy
    xn = nc.declare_dram_parameter("xn", [128, NT * D], f32r, False)
    m0 = nc.declare_dram_parameter("m0", [128, NB * SLOT], f32r, False)
    # crossing-segment tails only ever occupy the first M1R rows of a tile
    m1 = nc.declare_dram_parameter("m1", [M1R, NB * SLOT], f32r, False)
    OG = 8                      # out blocks per store DMA
    mo = nc.declare_dram_parameter("mout", [SLOT, NB * D], f32, True)

    XCH = ((0, 12), (12, 24), (24, 33))     # x load chunks (tile ranges)

    with tile.TileContext(nc) as tc:
        with ExitStack() as ctx:
            xp = ctx.enter_context(tc.tile_pool(name="x", bufs=1))
            mp = ctx.enter_context(tc.tile_pool(name="m", bufs=1))
            pp = ctx.enter_context(
                tc.tile_pool(name="p", bufs=6, space=bass.MemorySpace.PSUM))

            xt = xp.tile([128, NT, D], f32r, tag="x")
            nc.sync.dma_start(xt[:, XCH[0][0]:XCH[0][1], :],
                              xn[:, XCH[0][0] * D:XCH[0][1] * D])
            m0t = mp.tile([128, NB, SLOT], f32r, tag="m0")
            nc.sync.dma_start(m0t[:], m0[:, :])
            m1t = mp.tile([M1R, NB, SLOT], f32r, tag="m1")
            nc.sync.dma_start(m1t[:], m1[:, :])
            for c0, c1 in XCH[1:]:
                nc.sync.dma_start(xt[:, c0:c1, :], xn[:, c0 * D:c1 * D])

            op = ctx.enter_context(tc.tile_pool(name="o", bufs=2))
            for og in range(NB // OG):
                ot = op.tile([SLOT, OG, D], f32, tag="o")
                for k in range(OG):
                    i = og * OG + k
                    ps = pp.tile([SLOT, D], f32, tag="ps")
                    nc.tensor.matmul(ps[:], m0t[:, i, :], xt[:, i, :],
                                     start=True, stop=False)
                    nc.tensor.matmul(ps[:], m1t[:, i, :],
                                     xt[0:M1R, i + 1, :],
                                     start=False, stop=True)
                    if k % 2 == 0:
                        nc.vector.tensor_copy(ot[:, k, :], ps[:])
                    else:
                        nc.scalar.activation(ot[:, k, :], ps[:], AF.Copy)
                nc.sync.dma_start(mo[:, og * OG * D:(og + 1) * OG * D], ot[:])

    nc.finalize()
    _NC_CACHE["p2"] = nc
    return nc


# ---------------------------------------------------------------- host math

def _round_f32r(a):
    """Round f32 array to the fp32r grid (RNE at mantissa bit 12) so plain
    DMAs satisfy the verifier's 'rounded to FP32r' producer requirement."""
    b = np.ascontiguousarray(a, dtype=np.float32).view(np.uint32).astype(np.uint64)
    lsb = (b >> np.uint64(12)) & np.uint64(1)
    b = (b + np.uint64(0x7FF) + lsb) & ~np.uint64(0xFFF)
    return b.astype(np.uint32).view(np.float32)


def _sigmoid_np(v):
    v64 = v.astype(np.float64)
    return (1.0 / (1.0 + np.exp(-v64))).astype(np.float32)


def _quantile07_np(row):
    s = np.sort(row)
    qq = np.float32(0.7) * np.float32(L - 1)
    lo = int(np.floor(qq))
    hi = int(np.ceil(qq))
    hw = np.float32(qq - np.float32(lo))
    lw = np.float32(np.float32(1.0) - hw)
    return np.float32(s[lo] * lw + s[hi] * hw)


def _exact_signal_tokens(xr, mrow, toks, W1, b1, W2, b2, CL, CC, CR,
                         bb1eff, Wb2, bb2, sigmoid):
    """Exact f32 score pipeline for selected tokens of one row."""
    n = len(toks)
    xc = xr[toks]
    xl = np.zeros_like(xc)
    ok = toks - 1 >= 0
    xl[ok] = xr[toks[ok] - 1]
    xg = np.zeros_like(xc)
    ok = toks + 1 < L
    xg[ok] = xr[toks[ok] + 1]
    h1 = np.zeros((n, 128), np.float32)
    h2 = np.zeros((n, 128), np.float32)
    for k0 in range(0, D, 128):
        s = slice(k0, k0 + 128)
        h1 += xc[:, s] @ W1[s]
        h2 += xc[:, s] @ CC[s]
        h2 += xl[:, s] @ CL[s]
        h2 += xg[:, s] @ CR[s]
    l1 = np.maximum(h1 + b1, 0) @ W2 + b2
    l2 = np.maximum(h2 + bb1eff, 0) @ Wb2 + bb2
    content = sigmoid(l1[:, 0])
    bscore = sigmoid(l2[:, 0])
    return (content * (np.float32(1.0) - bscore) * mrow[toks]).astype(np.float32)


def _enforce_row(cand):
    """Replicates reference._enforce for one row. cand: (L,) bool."""
    cpos = np.flatnonzero(cand)
    acc_pos = []
    acc_start = []
    acc_sz = []
    acc_k = []
    start = 0
    for p in cpos:
        ps = p - start
        if ps >= MIN_PS:
            k = (ps + MAX_PS - 1) // MAX_PS
            sz = max(ps // k, 1)
            acc_pos.append(p)
            acc_start.append(start)
            acc_sz.append(sz)
            acc_k.append(k)
            start = p
    new_b = np.zeros(L, dtype=bool)
    ap = np.asarray(acc_pos, dtype=np.int64)
    new_b[ap] = True
    if len(ap):
        a_s = np.asarray(acc_start, dtype=np.int64)
        a_z = np.asarray(acc_sz, dtype=np.int64)
        a_k = np.asarray(acc_k, dtype=np.int64)
        pos = np.arange(L, dtype=np.int64)
        nxt = np.searchsorted(ap, pos, side="right")
        fv = nxt < len(ap)
        nxt_c = np.minimum(nxt, len(ap) - 1)
        fs = a_s[nxt_c]
        fz = a_z[nxt_c]
        fk = a_k[nxt_c]
        r = pos - fs
        j = r // fz
        split = fv & (r > 0) & (r % fz == 0) & (j >= 1) & (j <= fk - 1)
        new_b |= split
    return new_b


def _install_profile_shim():
    """The agent image lacks antenv.axon_hooks; recreate it from trn_boot's
    ctypes plumbing so run_bass_kernel_spmd(trace=True) can capture NTFF."""
    import sys, types
    if "antenv.axon_hooks" in sys.modules:
        return
    try:
        from trn_agent_boot import trn_boot
        hook = trn_boot._ntff_profile_via_ctypes("/opt/axon/libaxon_pjrt.so")
    except Exception:
        hook = None
    mod = types.ModuleType("antenv.axon_hooks")
    mod._hook = hook
    mod.get_axon_ntff_profile_hook = lambda: mod._hook
    def _set(h):
        mod._hook = h
    mod.set_axon_ntff_profile_hook = _set
    sys.modules["antenv.axon_hooks"] = mod


def _run(bass_utils, nc, in_maps, core_ids, label):
    import tempfile
    if PROFILE:
        _install_profile_shim()
        tmpdir = tempfile.mkdtemp(prefix=f"atm_{label}_")
        res = bass_utils.run_bass_kernel_spmd(
            nc, in_maps, core_ids, trace=True, tmpdir=tmpdir)
        LAST_EXEC_NS[label] = res.exec_time_ns
        LAST_TRACE_DIRS[label] = tmpdir
        return res
    return bass_utils.run_bass_kernel_spmd(nc, in_maps, core_ids)


# ---------------------------------------------------------------- kernel

def kernel(x, attention_mask, W1, b1, W2, b2, Wc, bc, Wb1, bb1, Wb2, bb2):
    jax = _get_jax()
    import jax.numpy as jnp
    from concourse import bass_utils

    x = np.ascontiguousarray(np.asarray(x, dtype=np.float32))
    attention_mask = np.asarray(attention_mask, dtype=np.float32)
    W1 = np.asarray(W1, np.float32); b1 = np.asarray(b1, np.float32)
    W2 = np.asarray(W2, np.float32); b2 = np.asarray(b2, np.float32)
    Wc = np.asarray(Wc, np.float32); bc = np.asarray(bc, np.float32)
    Wb1 = np.asarray(Wb1, np.float32); bb1 = np.asarray(bb1, np.float32)
    Wb2 = np.asarray(Wb2, np.float32); bb2 = np.asarray(bb2, np.float32)

    cpu = _cpu_dev(jax)
    core_ids = list(range(NCORES))

    # ---- weight prep: fold grouped conv into 3 dense (D,128) mats
    V = Wb1[D:, :]                                      # (256, 128)
    dd = np.arange(D)
    g = dd // 4
    ci = dd % 4
    A = np.zeros((3, D, 128), np.float32)
    for k in range(3):
        for j in range(2):
            o_idx = 2 * g + j
            A[k] += (Wc[o_idx, ci, k][:, None] * V[o_idx]).astype(np.float32)
    CC = (Wb1[:D, :] + A[1]).astype(np.float32)
    CL, CR = A[0], A[2]
    bb1eff = (bb1 + bc @ V).astype(np.float32)

    wmats = np.zeros((16, 128, 128), np.float32)
    for kc in range(4):
        sl = slice(kc * 128, (kc + 1) * 128)
        wmats[kc] = W1[sl]
        wmats[4 + kc] = CC[sl]
        wmats[8 + kc] = CL[sl]
        wmats[12 + kc] = CR[sl]
    vecs = np.stack([W2.reshape(128), Wb2.reshape(128), b1.reshape(128),
                     bb1eff.reshape(128)], axis=1).astype(np.float32)
    vecs = np.ascontiguousarray(vecs)
    wmats_r = _round_f32r(wmats)
    vecsr = _round_f32r(np.stack([W2.reshape(128), Wb2.reshape(128)], axis=1))

    # ---- phase-1 shard inputs
    in_maps1 = []
    for c in range(NCORES):
        r, h = c // 2, c % 2
        t0 = h * HALF
        xTh = np.zeros((D, TS + 2), np.float32)
        n_real = min(TS, L - t0)
        xTh[:, 1:1 + n_real] = x[r, t0:t0 + n_real].T
        if t0 > 0:
            xTh[:, 0] = x[r, t0 - 1]
        if t0 + TS < L:
            xTh[:, TS + 1] = x[r, t0 + TS]
        in_maps1.append({"xT": _round_f32r(xTh), "wmats": wmats_r,
                         "vecs": vecs, "vecsr": vecsr})

    nc1 = _build_phase1()
    r1 = _run(bass_utils, nc1, in_maps1, core_ids, "phase1")
    res1 = r1.results

    logit1 = np.zeros((B, L), np.float32)
    logit2 = np.zeros((B, L), np.float32)
    for c in range(NCORES):
        r, h = c // 2, c % 2
        t0 = h * HALF
        logit1[r, t0:t0 + HALF] = res1[c]["l1"][0, :HALF]
        logit2[r, t0:t0 + HALF] = res1[c]["l2"][0, :HALF]

    # ---- boundary decisions: approx (fp32r device) signal + exact rescue of
    # tokens near the per-row threshold.
    if cpu is not None:
        def sigmoid(v):
            with jax.default_device(cpu):
                return np.asarray(jax.nn.sigmoid(jnp.asarray(
                    np.ascontiguousarray(v, np.float32))))
    else:
        sigmoid = _sigmoid_np

    contentA = sigmoid(logit1 + b2[0])
    bscoreA = sigmoid(logit2 + bb2[0])
    sigA = (contentA * (np.float32(1.0) - bscoreA) * attention_mask
            ).astype(np.float32)

    DELTA = np.float32(2e-3)
    qidx = np.float32(0.7) * np.float32(L - 1)     # replicate jnp.quantile f32
    qlo = int(np.floor(qidx))
    qhi = int(np.ceil(qidx))
    qhw = np.float32(qidx - np.float32(qlo))
    qlw = np.float32(np.float32(1.0) - qhw)

    mb = np.zeros((B, L), dtype=bool)
    for r in range(B):
        sA = sigA[r]
        thrA = np.float32(np.quantile(sA.astype(np.float64), 0.7))
        dev = sA - thrA
        band = np.abs(dev) <= DELTA
        below_out = dev < -DELTA
        toks = np.flatnonzero(band)
        ok = False
        if len(toks):
            sigx = _exact_signal_tokens(
                x[r], attention_mask[r], toks, W1, b1, W2, b2, CL, CC, CR,
                bb1eff, Wb2, bb2, sigmoid)
            n_below = int(below_out.sum())
            i_lo, i_hi = qlo - n_below, qhi - n_below
            err = np.abs(sigx - sA[toks]).max()
            if err <= DELTA / 4 and 0 <= i_lo and i_hi < len(toks):
                svals = np.sort(sigx)
                # the chosen order stats must sit well inside the band so
                # approx-vs-exact rank swaps at the band edges cannot reach them
                edge_gap = min(svals[i_lo] - (thrA - DELTA),
                               (thrA + DELTA) - svals[i_hi])
                if edge_gap > 8 * max(err, np.float32(1e-7)):
                    thr = np.float32(svals[i_lo] * qlw + svals[i_hi] * qhw)
                    row_mb = below_out.copy()
                    row_mb[toks] = sigx < thr
                    mb[r] = row_mb
                    ok = True
        if not ok:
            # fallback: exact scores for every token of the row
            sigx = _exact_signal_tokens(
                x[r], attention_mask[r], np.arange(L), W1, b1, W2, b2,
                CL, CC, CR, bb1eff, Wb2, bb2, sigmoid)
            svals = np.sort(sigx)
            thr = np.float32(svals[qlo] * qlw + svals[qhi] * qhw)
            mb[r] = sigx < thr

    pos = np.arange(L)
    cand = mb | (pos[None, :] == L - 1)

    # ---- enforce + segment scalars + M matrices
    xm = x * attention_mask[..., None]
    e = np.einsum("bld,bld->bl", xm, xm).astype(np.float32)

    M0 = np.zeros((NCORES, NB, 128, SLOT), np.float32)
    M1 = np.zeros((NCORES, NB, M1R, SLOT), np.float32)
    # per row: mapping info to assemble output later
    row_maps = []
    n_last = np.zeros(B, np.int64)
    for r in range(B):
        new_b = _enforce_row(cand[r])
        seg = np.cumsum(new_b.astype(np.int64))        # inclusive cumsum
        n_last[r] = seg[-1]
        starts = np.concatenate([[0], np.flatnonzero(new_b)])
        nseg = len(starts)
        # softmax weights per token (f32, replicating reference ops)
        er = e[r]
        m_seg = np.maximum.reduceat(er, starts)
        w = np.exp(er - m_seg[seg]).astype(np.float32)
        denom = np.add.reduceat(w.astype(np.float32), starts)
        wn = (w / denom[seg]).astype(np.float32)
        wnm = wn * attention_mask[r]

        sj = starts                                     # (nseg,)
        hseg = (sj >= HALF).astype(np.int64)
        cseg = 2 * r + hseg
        t0seg = hseg * HALF
        itile = (sj - t0seg) >> 7
        key = cseg * NB + itile
        first_idx = np.zeros(len(key), np.int64)
        uk, fidx = np.unique(key, return_index=True)
        # segments sorted by start => key nondecreasing => rank within key:
        kmap = {int(kk): int(fi) for kk, fi in zip(uk, fidx)}
        first = np.asarray([kmap[int(kk)] for kk in key], np.int64)
        slot = np.arange(nseg) - first

        # token-level placement
        tpos = np.arange(L)
        jtok = seg                                      # segment idx per token
        s_t = sj[jtok]
        h_t = hseg[jtok]
        c_t = cseg[jtok]
        t0_t = t0seg[jtok]
        it_t = itile[jtok]
        slot_t = slot[jtok]
        tl = tpos - t0_t
        which = tl >> 7
        in_m1 = which != it_t                           # crossing into tile+1
        # local row inside the (128,) partition dim of the M block
        loc0 = tl - (it_t << 7)
        loc1 = tl - ((it_t + 1) << 7)
        sel0 = ~in_m1
        M0[c_t[sel0], it_t[sel0], loc0[sel0], slot_t[sel0]] = wnm[sel0]
        sel1 = in_m1
        M1[c_t[sel1], it_t[sel1], loc1[sel1], slot_t[sel1]] = wnm[sel1]

        row_maps.append((cseg, itile * SLOT + slot))

    # ---- phase-2 shard inputs
    in_maps2 = []
    for c in range(NCORES):
        r, h = c // 2, c % 2
        t0 = h * HALF
        xnw = np.zeros((TS, D), np.float32)
        n_real = min(TS, L - t0)
        xnw[:n_real] = x[r, t0:t0 + n_real]
        xnt = xnw.reshape(NT, 128, D).transpose(1, 0, 2).reshape(128, NT * D)
        m0t = M0[c].transpose(1, 0, 2).reshape(128, NB * SLOT)
        m1t = M1[c].transpose(1, 0, 2).reshape(M1R, NB * SLOT)
        in_maps2.append({"xn": _round_f32r(xnt),
                         "m0": _round_f32r(m0t),
                         "m1": _round_f32r(m1t)})

    nc2 = _build_phase2()
    r2 = _run(bass_utils, nc2, in_maps2, core_ids, "phase2")
    res2 = r2.results

    # ---- assemble outputs
    merged = np.zeros((B, L, D), np.float32)
    for r in range(B):
        cseg, localrow = row_maps[r]
        jglob = np.arange(len(cseg))
        for c in (2 * r, 2 * r + 1):
            selc = cseg == c
            if selc.any():
                mo_c = res2[c]["mout"].reshape(SLOT, NB, D).transpose(1, 0, 2)
                merged[r, jglob[selc]] = mo_c.reshape(NB * SLOT, D)[localrow[selc]]

    seg_bounds = ((pos[None, :] >= 1) &
                  (pos[None, :] <= n_last[:, None])).astype(np.float32)
    return merged, seg_bounds


# revision 36
# speedup vs baseline: 1.1616x; 1.1616x over previous
"""AdaptiveTokenMerger Trainium2 kernel (8 NeuronCores, SPMD).

Decomposition:
  Phase 1 (device): per-token score logits. The grouped conv1d is folded
    algebraically into 3 dense shifted matmuls (Ak weight prep on host), so
    phase 1 is pure f32 PE matmuls + exact ReLU. Outputs pre-sigmoid logits.
  Host: sigmoids/quantile via jax-CPU (bitwise-matching the reference's
    boundary decisions), the inherently sequential boundary-enforce walk,
    per-segment softmax scalars, and the block-compaction weight matrices M.
  Phase 2 (device): merged vectors = M^T @ x per 128-token tile (f32 PE
    matmuls, PSUM accumulation across the tile-crossing part). Output is
    block-compacted; host permutes rows into the final (B, L, D) layout.

Sharding: pure data parallel over (batch row, half sequence) = 8 shards.
"""

import numpy as np
from contextlib import ExitStack

B, L, D = 4, 8192, 512
NCORES = 8
HALF = L // 2            # 4096 tokens per shard (plus halo/padding)
NT = 33                  # 128-token x tiles per shard window
TS = NT * 128            # 4224 padded window length
NB = NT - 1              # 32 output blocks per shard
GRP = 384                # phase-1 matmul moving free dim (11 * 384 = 4224)
NGRP = TS // GRP
M1R = 20                 # rows of a tile reachable by a crossing-segment tail
SLOT = 32                # output slots per 128-token tile (hard bound: 128/MIN_PS)
MIN_PS, MAX_PS = 4, 16

_NC_CACHE = {}
PROFILE = False          # set True (e.g. from test.py) to capture NTFF exec times
LAST_EXEC_NS = {}        # phase name -> exec_time_ns (when PROFILE)
LAST_TRACE_DIRS = {}


def _get_jax():
    import jax
    try:
        jax.config.update("jax_platforms", "axon,cpu")
    except Exception:
        pass
    return jax


def _cpu_dev(jax):
    try:
        return jax.devices("cpu")[0]
    except Exception:
        return None


# ---------------------------------------------------------------- builders

def _build_phase1():
    if "p1" in _NC_CACHE:
        return _NC_CACHE["p1"]
    from concourse import bacc, mybir
    import concourse.tile as tile
    import concourse.bass as bass

    f32 = mybir.dt.float32
    f32r = mybir.dt.float32r
    AF = mybir.ActivationFunctionType
    ALU = mybir.AluOpType
    nc = bacc.Bacc(None, target_bir_lowering=False, debug=False)
    # xT / wmats / vecsr arrive pre-rounded to the fp32r grid (host _round_f32r)
    xT = nc.declare_dram_parameter("xT", [D, TS + 2], f32r, False)
    wmats = nc.declare_dram_parameter("wmats", [16, 128, 128], f32r, False)
    vecs = nc.declare_dram_parameter("vecs", [128, 4], f32, False)
    vecsr = nc.declare_dram_parameter("vecsr", [128, 2], f32r, False)
    l1o = nc.declare_dram_parameter("l1", [1, TS], f32, True)
    l2o = nc.declare_dram_parameter("l2", [1, TS], f32, True)

    # token groups: 8 x 512 + tail 128
    GROUPS = [(i * 512, 512) for i in range(8)] + [(4096, 128)]

    with tile.TileContext(nc) as tc:
        with ExitStack() as ctx:
            xtp = ctx.enter_context(tc.tile_pool(name="xt", bufs=1))
            wtp = ctx.enter_context(tc.tile_pool(name="wt", bufs=1))
            hsp = ctx.enter_context(tc.tile_pool(name="hs", bufs=3))
            lsp = ctx.enter_context(tc.tile_pool(name="ls", bufs=3))
            psp = ctx.enter_context(
                tc.tile_pool(name="ps", bufs=3, space=bass.MemorySpace.PSUM))
            pslp = ctx.enter_context(
                tc.tile_pool(name="psl", bufs=1, space=bass.MemorySpace.PSUM))

            vt = wtp.tile([128, 4], f32, tag="vecs")
            nc.sync.dma_start(vt[:], vecs[:, :])
            vr = wtp.tile([128, 2], f32r, tag="vecsr")
            nc.sync.dma_start(vr[:], vecsr[:, :])
            wts = []
            for i in range(16):
                t = wtp.tile([128, 128], f32r, tag=f"w{i}")
                nc.sync.dma_start(t[:], wmats[i, :, :])
                wts.append(t)
            w1t, cct, clt, crt = wts[0:4], wts[4:8], wts[8:12], wts[12:16]
            xts = []
            for kc in range(4):
                xtile = xtp.tile([128, TS + 2], f32r, tag=f"x{kc}")
                xts.append(xtile)
            CB = [0, 516, 1028, 1540, 2052, 2564, 3076, 3588, 4128, 4226]
            for j in range(len(CB) - 1):
                for kc in range(4):
                    nc.sync.dma_start(
                        xts[kc][:, CB[j]:CB[j + 1]],
                        xT[kc * 128:(kc + 1) * 128, CB[j]:CB[j + 1]])

            for o, gs in GROUPS:
                h1p = psp.tile([128, 512], f32, tag="h1")
                for kc in range(4):
                    nc.tensor.matmul(h1p[:, 0:gs], w1t[kc][:],
                                     xts[kc][:, 1 + o:1 + o + gs],
                                     start=(kc == 0), stop=(kc == 3))
                h1s = hsp.tile([128, 512], f32r, tag="h1s")
                nc.scalar.activation(h1s[:, 0:gs], h1p[:, 0:gs],
                                     AF.Relu, bias=vt[:, 2:3])
                l1p = pslp.tile([1, 512], f32, tag="l1p")
                nc.tensor.matmul(l1p[:, 0:gs], vr[:, 0:1], h1s[:, 0:gs],
                                 start=True, stop=True)
                l1t = lsp.tile([1, 512], f32, tag="l1t")
                nc.vector.tensor_copy(l1t[:, 0:gs], l1p[:, 0:gs])
                nc.gpsimd.dma_start(l1o[:, o:o + gs], l1t[:, 0:gs])

                h2p = psp.tile([128, 512], f32, tag="h2")
                idx = 0
                for kc in range(4):
                    for mats, off in ((cct, 1), (clt, 0), (crt, 2)):
                        nc.tensor.matmul(h2p[:, 0:gs], mats[kc][:],
                                         xts[kc][:, off + o:off + o + gs],
                                         start=(idx == 0), stop=(idx == 11))
                        idx += 1
                h2s = hsp.tile([128, 512], f32r, tag="h2s")
                nc.vector.tensor_scalar(h2s[:, 0:gs], h2p[:, 0:gs],
                                        vt[:, 3:4], 0.0, ALU.add, ALU.max)
                l2p = pslp.tile([1, 512], f32, tag="l2p")
                nc.tensor.matmul(l2p[:, 0:gs], vr[:, 1:2], h2s[:, 0:gs],
                                 start=True, stop=True)
                l2t = lsp.tile([1, 512], f32, tag="l2t")
                nc.scalar.activation(l2t[:, 0:gs], l2p[:, 0:gs], AF.Copy)
                nc.gpsimd.dma_start(l2o[:, o:o + gs], l2t[:, 0:gs])

    nc.finalize()
    _NC_CACHE["p1"] = nc
    return nc


def _build_phase2():
    if "p2" in _NC_CACHE:
        return _NC_CACHE["p2"]
    from concourse import bacc, mybir
    import concourse.tile as tile
    import concourse.bass as bass

    f32 = mybir.dt.float32
    f32r = mybir.dt.float32r
    AF = mybir.ActivationFunctionType
    nc = bacc.Bacc(None, target_bir_lowering=False, debug=False)
    # host supplies pre-transposed, fp32r-rounded layouts so every DMA is a
    # plain 2D partition-contiguous copy
    xn = nc.declare_dram_parameter("xn", [128, NT * D], f32r, False)
    m0 = nc.declare_dram_parameter("m0", [128, NB * SLOT], f32r, False)
    # crossing-segment tails only ever occupy the first M1R rows of a tile
    m1 = nc.declare_dram_parameter("m1", [128, NB * SLOT], f32r, False)
    OG = 4                      # out blocks per store DMA
    mo = nc.declare_dram_parameter("mout", [SLOT, NB * D], f32, True)

    XCH = ((0, 6), (6, 12), (12, 18), (18, 24), (24, 29), (29, 33))

    with tile.TileContext(nc) as tc:
        with ExitStack() as ctx:
            xp = ctx.enter_context(tc.tile_pool(name="x", bufs=1))
            mp = ctx.enter_context(tc.tile_pool(name="m", bufs=1))
            pp = ctx.enter_context(
                tc.tile_pool(name="p", bufs=6, space=bass.MemorySpace.PSUM))

            xt = xp.tile([128, NT, D], f32r, tag="x")
            nc.sync.dma_start(xt[:, XCH[0][0]:XCH[0][1], :],
                              xn[:, XCH[0][0] * D:XCH[0][1] * D])
            m0t = mp.tile([128, NB, SLOT], f32r, tag="m0")
            nc.sync.dma_start(m0t[:], m0[:, :])
            m1t = mp.tile([128, NB, SLOT], f32r, tag="m1")
            nc.sync.dma_start(m1t[:], m1[:, :])
            for c0, c1 in XCH[1:]:
                nc.sync.dma_start(xt[:, c0:c1, :], xn[:, c0 * D:c1 * D])

            op = ctx.enter_context(tc.tile_pool(name="o", bufs=3))
            for og in range(NB // OG):
                ot = op.tile([SLOT, OG, D], f32, tag="o")
                for k in range(OG):
                    i = og * OG + k
                    ps = pp.tile([SLOT, D], f32, tag="ps")
                    nc.tensor.matmul(ps[:], m0t[:, i, :], xt[:, i, :],
                                     start=True, stop=False)
                    nc.tensor.matmul(ps[:], m1t[:, i, :],
                                     xt[:, i + 1, :],
                                     start=False, stop=True)
                    if k % 2 == 0:
                        nc.vector.tensor_copy(ot[:, k, :], ps[:])
                    else:
                        nc.scalar.activation(ot[:, k, :], ps[:], AF.Copy)
                nc.sync.dma_start(mo[:, og * OG * D:(og + 1) * OG * D], ot[:])

    nc.finalize()
    _NC_CACHE["p2"] = nc
    return nc


# ---------------------------------------------------------------- host math

def _round_f32r(a):
    """Round f32 array to the fp32r grid (RNE at mantissa bit 12) so plain
    DMAs satisfy the verifier's 'rounded to FP32r' producer requirement."""
    b = np.ascontiguousarray(a, dtype=np.float32).view(np.uint32).astype(np.uint64)
    lsb = (b >> np.uint64(12)) & np.uint64(1)
    b = (b + np.uint64(0x7FF) + lsb) & ~np.uint64(0xFFF)
    return b.astype(np.uint32).view(np.float32)


def _sigmoid_np(v):
    v64 = v.astype(np.float64)
    return (1.0 / (1.0 + np.exp(-v64))).astype(np.float32)


def _quantile07_np(row):
    s = np.sort(row)
    qq = np.float32(0.7) * np.float32(L - 1)
    lo = int(np.floor(qq))
    hi = int(np.ceil(qq))
    hw = np.float32(qq - np.float32(lo))
    lw = np.float32(np.float32(1.0) - hw)
    return np.float32(s[lo] * lw + s[hi] * hw)


def _exact_signal_tokens(xr, mrow, toks, W1, b1, W2, b2, CL, CC, CR,
                         bb1eff, Wb2, bb2, sigmoid):
    """Exact f32 score pipeline for selected tokens of one row."""
    n = len(toks)
    xc = xr[toks]
    xl = np.zeros_like(xc)
    ok = toks - 1 >= 0
    xl[ok] = xr[toks[ok] - 1]
    xg = np.zeros_like(xc)
    ok = toks + 1 < L
    xg[ok] = xr[toks[ok] + 1]
    h1 = np.zeros((n, 128), np.float32)
    h2 = np.zeros((n, 128), np.float32)
    for k0 in range(0, D, 128):
        s = slice(k0, k0 + 128)
        h1 += xc[:, s] @ W1[s]
        h2 += xc[:, s] @ CC[s]
        h2 += xl[:, s] @ CL[s]
        h2 += xg[:, s] @ CR[s]
    l1 = np.maximum(h1 + b1, 0) @ W2 + b2
    l2 = np.maximum(h2 + bb1eff, 0) @ Wb2 + bb2
    content = sigmoid(l1[:, 0])
    bscore = sigmoid(l2[:, 0])
    return (content * (np.float32(1.0) - bscore) * mrow[toks]).astype(np.float32)


def _enforce_row(cand):
    """Replicates reference._enforce for one row. cand: (L,) bool."""
    cpos = np.flatnonzero(cand)
    acc_pos = []
    acc_start = []
    acc_sz = []
    acc_k = []
    start = 0
    for p in cpos:
        ps = p - start
        if ps >= MIN_PS:
            k = (ps + MAX_PS - 1) // MAX_PS
            sz = max(ps // k, 1)
            acc_pos.append(p)
            acc_start.append(start)
            acc_sz.append(sz)
            acc_k.append(k)
            start = p
    new_b = np.zeros(L, dtype=bool)
    ap = np.asarray(acc_pos, dtype=np.int64)
    new_b[ap] = True
    if len(ap):
        a_s = np.asarray(acc_start, dtype=np.int64)
        a_z = np.asarray(acc_sz, dtype=np.int64)
        a_k = np.asarray(acc_k, dtype=np.int64)
        pos = np.arange(L, dtype=np.int64)
        nxt = np.searchsorted(ap, pos, side="right")
        fv = nxt < len(ap)
        nxt_c = np.minimum(nxt, len(ap) - 1)
        fs = a_s[nxt_c]
        fz = a_z[nxt_c]
        fk = a_k[nxt_c]
        r = pos - fs
        j = r // fz
        split = fv & (r > 0) & (r % fz == 0) & (j >= 1) & (j <= fk - 1)
        new_b |= split
    return new_b


def _install_profile_shim():
    """The agent image lacks antenv.axon_hooks; recreate it from trn_boot's
    ctypes plumbing so run_bass_kernel_spmd(trace=True) can capture NTFF."""
    import sys, types
    if "antenv.axon_hooks" in sys.modules:
        return
    try:
        from trn_agent_boot import trn_boot
        hook = trn_boot._ntff_profile_via_ctypes("/opt/axon/libaxon_pjrt.so")
    except Exception:
        hook = None
    mod = types.ModuleType("antenv.axon_hooks")
    mod._hook = hook
    mod.get_axon_ntff_profile_hook = lambda: mod._hook
    def _set(h):
        mod._hook = h
    mod.set_axon_ntff_profile_hook = _set
    sys.modules["antenv.axon_hooks"] = mod


def _run(bass_utils, nc, in_maps, core_ids, label):
    import tempfile
    if PROFILE:
        _install_profile_shim()
        tmpdir = tempfile.mkdtemp(prefix=f"atm_{label}_")
        res = bass_utils.run_bass_kernel_spmd(
            nc, in_maps, core_ids, trace=True, tmpdir=tmpdir)
        LAST_EXEC_NS[label] = res.exec_time_ns
        LAST_TRACE_DIRS[label] = tmpdir
        return res
    return bass_utils.run_bass_kernel_spmd(nc, in_maps, core_ids)


# ---------------------------------------------------------------- kernel

def kernel(x, attention_mask, W1, b1, W2, b2, Wc, bc, Wb1, bb1, Wb2, bb2):
    jax = _get_jax()
    import jax.numpy as jnp
    from concourse import bass_utils

    x = np.ascontiguousarray(np.asarray(x, dtype=np.float32))
    attention_mask = np.asarray(attention_mask, dtype=np.float32)
    W1 = np.asarray(W1, np.float32); b1 = np.asarray(b1, np.float32)
    W2 = np.asarray(W2, np.float32); b2 = np.asarray(b2, np.float32)
    Wc = np.asarray(Wc, np.float32); bc = np.asarray(bc, np.float32)
    Wb1 = np.asarray(Wb1, np.float32); bb1 = np.asarray(bb1, np.float32)
    Wb2 = np.asarray(Wb2, np.float32); bb2 = np.asarray(bb2, np.float32)

    cpu = _cpu_dev(jax)
    core_ids = list(range(NCORES))

    # ---- weight prep: fold grouped conv into 3 dense (D,128) mats
    V = Wb1[D:, :]                                      # (256, 128)
    dd = np.arange(D)
    g = dd // 4
    ci = dd % 4
    A = np.zeros((3, D, 128), np.float32)
    for k in range(3):
        for j in range(2):
            o_idx = 2 * g + j
            A[k] += (Wc[o_idx, ci, k][:, None] * V[o_idx]).astype(np.float32)
    CC = (Wb1[:D, :] + A[1]).astype(np.float32)
    CL, CR = A[0], A[2]
    bb1eff = (bb1 + bc @ V).astype(np.float32)

    wmats = np.zeros((16, 128, 128), np.float32)
    for kc in range(4):
        sl = slice(kc * 128, (kc + 1) * 128)
        wmats[kc] = W1[sl]
        wmats[4 + kc] = CC[sl]
        wmats[8 + kc] = CL[sl]
        wmats[12 + kc] = CR[sl]
    vecs = np.stack([W2.reshape(128), Wb2.reshape(128), b1.reshape(128),
                     bb1eff.reshape(128)], axis=1).astype(np.float32)
    vecs = np.ascontiguousarray(vecs)
    wmats_r = _round_f32r(wmats)
    vecsr = _round_f32r(np.stack([W2.reshape(128), Wb2.reshape(128)], axis=1))

    # ---- phase-1 shard inputs
    in_maps1 = []
    for c in range(NCORES):
        r, h = c // 2, c % 2
        t0 = h * HALF
        xTh = np.zeros((D, TS + 2), np.float32)
        n_real = min(TS, L - t0)
        xTh[:, 1:1 + n_real] = x[r, t0:t0 + n_real].T
        if t0 > 0:
            xTh[:, 0] = x[r, t0 - 1]
        if t0 + TS < L:
            xTh[:, TS + 1] = x[r, t0 + TS]
        in_maps1.append({"xT": _round_f32r(xTh), "wmats": wmats_r,
                         "vecs": vecs, "vecsr": vecsr})

    nc1 = _build_phase1()
    r1 = _run(bass_utils, nc1, in_maps1, core_ids, "phase1")
    res1 = r1.results

    logit1 = np.zeros((B, L), np.float32)
    logit2 = np.zeros((B, L), np.float32)
    for c in range(NCORES):
        r, h = c // 2, c % 2
        t0 = h * HALF
        logit1[r, t0:t0 + HALF] = res1[c]["l1"][0, :HALF]
        logit2[r, t0:t0 + HALF] = res1[c]["l2"][0, :HALF]

    # ---- boundary decisions: approx (fp32r device) signal + exact rescue of
    # tokens near the per-row threshold.
    if cpu is not None:
        def sigmoid(v):
            with jax.default_device(cpu):
                return np.asarray(jax.nn.sigmoid(jnp.asarray(
                    np.ascontiguousarray(v, np.float32))))
    else:
        sigmoid = _sigmoid_np

    contentA = sigmoid(logit1 + b2[0])
    bscoreA = sigmoid(logit2 + bb2[0])
    sigA = (contentA * (np.float32(1.0) - bscoreA) * attention_mask
            ).astype(np.float32)

    DELTA = np.float32(2e-3)
    qidx = np.float32(0.7) * np.float32(L - 1)     # replicate jnp.quantile f32
    qlo = int(np.floor(qidx))
    qhi = int(np.ceil(qidx))
    qhw = np.float32(qidx - np.float32(qlo))
    qlw = np.float32(np.float32(1.0) - qhw)

    mb = np.zeros((B, L), dtype=bool)
    for r in range(B):
        sA = sigA[r]
        thrA = np.float32(np.quantile(sA.astype(np.float64), 0.7))
        dev = sA - thrA
        band = np.abs(dev) <= DELTA
        below_out = dev < -DELTA
        toks = np.flatnonzero(band)
        ok = False
        if len(toks):
            sigx = _exact_signal_tokens(
                x[r], attention_mask[r], toks, W1, b1, W2, b2, CL, CC, CR,
                bb1eff, Wb2, bb2, sigmoid)
            n_below = int(below_out.sum())
            i_lo, i_hi = qlo - n_below, qhi - n_below
            err = np.abs(sigx - sA[toks]).max()
            if err <= DELTA / 4 and 0 <= i_lo and i_hi < len(toks):
                svals = np.sort(sigx)
                # the chosen order stats must sit well inside the band so
                # approx-vs-exact rank swaps at the band edges cannot reach them
                edge_gap = min(svals[i_lo] - (thrA - DELTA),
                               (thrA + DELTA) - svals[i_hi])
                if edge_gap > 8 * max(err, np.float32(1e-7)):
                    thr = np.float32(svals[i_lo] * qlw + svals[i_hi] * qhw)
                    row_mb = below_out.copy()
                    row_mb[toks] = sigx < thr
                    mb[r] = row_mb
                    ok = True
        if not ok:
            # fallback: exact scores for every token of the row
            sigx = _exact_signal_tokens(
                x[r], attention_mask[r], np.arange(L), W1, b1, W2, b2,
                CL, CC, CR, bb1eff, Wb2, bb2, sigmoid)
            svals = np.sort(sigx)
            thr = np.float32(svals[qlo] * qlw + svals[qhi] * qhw)
            mb[r] = sigx < thr

    pos = np.arange(L)
    cand = mb | (pos[None, :] == L - 1)

    # ---- enforce + segment scalars + M matrices
    xm = x * attention_mask[..., None]
    e = np.einsum("bld,bld->bl", xm, xm).astype(np.float32)

    M0 = np.zeros((NCORES, NB, 128, SLOT), np.float32)
    M1 = np.zeros((NCORES, NB, 128, SLOT), np.float32)
    # per row: mapping info to assemble output later
    row_maps = []
    n_last = np.zeros(B, np.int64)
    for r in range(B):
        new_b = _enforce_row(cand[r])
        seg = np.cumsum(new_b.astype(np.int64))        # inclusive cumsum
        n_last[r] = seg[-1]
        starts = np.concatenate([[0], np.flatnonzero(new_b)])
        nseg = len(starts)
        # softmax weights per token (f32, replicating reference ops)
        er = e[r]
        m_seg = np.maximum.reduceat(er, starts)
        w = np.exp(er - m_seg[seg]).astype(np.float32)
        denom = np.add.reduceat(w.astype(np.float32), starts)
        wn = (w / denom[seg]).astype(np.float32)
        wnm = wn * attention_mask[r]

        sj = starts                                     # (nseg,)
        hseg = (sj >= HALF).astype(np.int64)
        cseg = 2 * r + hseg
        t0seg = hseg * HALF
        itile = (sj - t0seg) >> 7
        key = cseg * NB + itile
        first_idx = np.zeros(len(key), np.int64)
        uk, fidx = np.unique(key, return_index=True)
        # segments sorted by start => key nondecreasing => rank within key:
        kmap = {int(kk): int(fi) for kk, fi in zip(uk, fidx)}
        first = np.asarray([kmap[int(kk)] for kk in key], np.int64)
        slot = np.arange(nseg) - first

        # token-level placement
        tpos = np.arange(L)
        jtok = seg                                      # segment idx per token
        s_t = sj[jtok]
        h_t = hseg[jtok]
        c_t = cseg[jtok]
        t0_t = t0seg[jtok]
        it_t = itile[jtok]
        slot_t = slot[jtok]
        tl = tpos - t0_t
        which = tl >> 7
        in_m1 = which != it_t                           # crossing into tile+1
        # local row inside the (128,) partition dim of the M block
        loc0 = tl - (it_t << 7)
        loc1 = tl - ((it_t + 1) << 7)
        sel0 = ~in_m1
        M0[c_t[sel0], it_t[sel0], loc0[sel0], slot_t[sel0]] = wnm[sel0]
        sel1 = in_m1
        M1[c_t[sel1], it_t[sel1], loc1[sel1], slot_t[sel1]] = wnm[sel1]

        row_maps.append((cseg, itile * SLOT + slot))

    # ---- phase-2 shard inputs
    in_maps2 = []
    for c in range(NCORES):
        r, h = c // 2, c % 2
        t0 = h * HALF
        xnw = np.zeros((TS, D), np.float32)
        n_real = min(TS, L - t0)
        xnw[:n_real] = x[r, t0:t0 + n_real]
        xnt = xnw.reshape(NT, 128, D).transpose(1, 0, 2).reshape(128, NT * D)
        m0t = M0[c].transpose(1, 0, 2).reshape(128, NB * SLOT)
        m1t = M1[c].transpose(1, 0, 2).reshape(128, NB * SLOT)
        in_maps2.append({"xn": _round_f32r(xnt),
                         "m0": _round_f32r(m0t),
                         "m1": _round_f32r(m1t)})

    nc2 = _build_phase2()
    r2 = _run(bass_utils, nc2, in_maps2, core_ids, "phase2")
    res2 = r2.results

    # ---- assemble outputs
    merged = np.zeros((B, L, D), np.float32)
    for r in range(B):
        cseg, localrow = row_maps[r]
        jglob = np.arange(len(cseg))
        for c in (2 * r, 2 * r + 1):
            selc = cseg == c
            if selc.any():
                mo_c = res2[c]["mout"].reshape(SLOT, NB, D).transpose(1, 0, 2)
                merged[r, jglob[selc]] = mo_c.reshape(NB * SLOT, D)[localrow[selc]]

    seg_bounds = ((pos[None, :] >= 1) &
                  (pos[None, :] <= n_last[:, None])).astype(np.float32)
    return merged, seg_bounds
